# revision 3
# baseline (speedup 1.0000x reference)
"""GCN layer kernel for Trainium2, 8 NeuronCores (SPMD).

Math (see reference):
    deg = scatter_add(ones, row); deg = max(deg, 1)
    norm_e = rsqrt(deg[row_e]) * rsqrt(deg[col_e])
    agg[row_e] += x[col_e] * norm_e
    out = agg @ W.T + b

Device strategy (v2):
  - Shard DESTINATION nodes across 8 cores (12500 each) -> no collective.
  - Edges sorted by (core, segment, chunk, dest). A segment is SEGW=4
    dest windows of WD=256 nodes. Per (segment, chunk) one large
    dma_gather call fetches x[src] rows (bf16, 256B rows) -- few calls
    amortize the ~1us SWDGE fixed cost; groups are padded to 128 edges
    only at (segment, chunk) granularity (~3% pad).
  - Scatter-add on the TensorEngine:
        aggT[f, d] += msgs[e, f]^T @ onehot[e, d]
    with onehot[e, d] = (d == dloc_e) * norm_e built by one fused DVE
    tensor_scalar (is_equal, mult) per 128-edge tile, all-SBUF bf16 (4x
    DVE mode), accumulating in PSUM over the window. Tiles may span
    window boundaries: a boundary tile is processed once per adjacent
    window with shifted dloc; foreign edges match no iota value and
    contribute exactly zero.
  - Epilogue per window: aggT PSUM -> SBUF bf16 (scalar engine), then
        outT[o, d] = W[o, :] @ aggT[:, d]
    as one N=256 bf16 matmul (lhsT = W.T), bias b[o] folded into the
    PSUM->SBUF copy via scalar.activation(Identity, bias=b). Output is
    stored feature-major [D, NPAD]; the host transposes at the end.

Host-side work is limited to index preprocessing (sort/shard/pad, int16
tables, degree/norm coefficients) and final unpadding/concat/transpose.
"""

import numpy as np
from contextlib import ExitStack

N_NODES = 100000
N_EDGES = 1600000
D = 128
NCORES = 8
NLOC = N_NODES // NCORES          # 12500 real dests per core
WD = 256                          # dest window
NWIN = (NLOC + WD - 1) // WD      # 49 windows (12544 padded dests)
NPAD = NWIN * WD                  # 12544
CHUNK = 32768                     # x chunk rows (int16 index range)
NCHUNK = (N_NODES + CHUNK - 1) // CHUNK  # 4
P = 128
SEGW = 4                          # windows per gather segment
NSEG = (NWIN + SEGW - 1) // SEGW  # 13


def _bf16():
    import ml_dtypes
    return np.dtype(ml_dtypes.bfloat16)


def _host_prep(x, edge_index, W, b):
    """Sort/shard/pad edges; build per-core device arrays + schedule meta."""
    row = np.asarray(edge_index[0], dtype=np.int64)
    col = np.asarray(edge_index[1], dtype=np.int64)

    deg = np.bincount(row, minlength=N_NODES).astype(np.float32)
    deg = np.maximum(deg, 1.0)
    rs = 1.0 / np.sqrt(deg)
    norm = (rs[row] * rs[col]).astype(np.float32)

    core = row // NLOC
    local = row - core * NLOC
    win = local // WD
    seg = win // SEGW
    chunk = col >> 15
    # sort by (core, seg, chunk, local-dest)
    order = np.lexsort((local, chunk, seg, core))
    col_s = col[order]
    local_s = local[order]
    norm_s = norm[order]

    gid = ((core * NSEG + seg) * NCHUNK + chunk)[order]
    ngroups = NCORES * NSEG * NCHUNK
    counts = np.bincount(gid, minlength=ngroups).reshape(NCORES, NSEG, NCHUNK)
    starts = np.zeros(ngroups + 1, dtype=np.int64)
    np.cumsum(counts.reshape(-1), out=starts[1:])

    # tiles per (seg, chunk): max over cores (shared NEFF), 128-edge tiles
    T_sc = np.ceil(counts.max(axis=0) / P).astype(np.int64)  # [NSEG, NCHUNK]
    total_tiles = int(T_sc.sum())
    total_padded = total_tiles * P
    # padded start of each (seg, chunk) group (same for all cores)
    gpad_start = np.zeros(NSEG * NCHUNK + 1, dtype=np.int64)
    np.cumsum((T_sc.reshape(-1) * P), out=gpad_start[1:])
    # tile offset of (seg, chunk) within its segment's msgs buffer
    seg_tile0 = np.zeros((NSEG, NCHUNK), dtype=np.int64)
    for s in range(NSEG):
        off = 0
        for c in range(NCHUNK):
            seg_tile0[s, c] = off
            off += int(T_sc[s, c])
    T_seg = T_sc.sum(axis=1)                     # [NSEG]
    T_seg_max = int(T_seg.max())

    # ---- per-core padded flats (dest-local, norm, int16 idx) ----
    loc_pad = np.full((NCORES, total_padded), -1.0, np.float32)
    nrm_pad = np.zeros((NCORES, total_padded), np.float32)
    ix_pad = np.zeros((NCORES, total_padded), np.int16)
    for k in range(NCORES):
        for s in range(NSEG):
            for c in range(NCHUNK):
                g = (k * NSEG + s) * NCHUNK + c
                e0, e1 = starts[g], starts[g + 1]
                n = int(e1 - e0)
                p0 = gpad_start[s * NCHUNK + c]
                loc_pad[k, p0:p0 + n] = local_s[e0:e1].astype(np.float32)
                nrm_pad[k, p0:p0 + n] = norm_s[e0:e1]
                ix_pad[k, p0:p0 + n] = (col_s[e0:e1] - (c << 15)).astype(np.int16)

    # ---- processing schedule (same for all cores): union tile ranges ----
    # window w, chunk c -> tiles [lo, hi) within group (seg(w), c)
    procs = []  # (w, seg, c, t_in_group)
    win_nproc = np.zeros(NWIN, np.int64)
    for s in range(NSEG):
        for w in range(s * SEGW, min((s + 1) * SEGW, NWIN)):
            for c in range(NCHUNK):
                t_cnt = int(T_sc[s, c])
                if t_cnt == 0:
                    continue
                lo_t, hi_t = t_cnt, 0
                for k in range(NCORES):
                    g = (k * NSEG + s) * NCHUNK + c
                    e0, e1 = starts[g], starts[g + 1]
                    lo_e = np.searchsorted(local_s[e0:e1], w * WD, side="left")
                    hi_e = np.searchsorted(local_s[e0:e1], (w + 1) * WD, side="left")
                    if hi_e > lo_e:
                        lo_t = min(lo_t, int(lo_e) // P)
                        hi_t = max(hi_t, -(-int(hi_e) // P))
                for t in range(lo_t, hi_t):
                    procs.append((w, s, c, t))
            win_nproc[w] = len(procs) - int(win_nproc[:w].sum())
    n_proc = len(procs)

    # ---- per-core consts: dloc | nrm per processing column ----
    consts = np.zeros((NCORES, P, 2 * n_proc), np.float32)
    for j, (w, s, c, t) in enumerate(procs):
        p0 = gpad_start[s * NCHUNK + c] + t * P
        sl = slice(p0, p0 + P)
        consts[:, :, j] = loc_pad[:, sl] - w * WD
        consts[:, :, n_proc + j] = nrm_pad[:, sl]

    # ---- idx table: wrapped in 16 partitions, replicated to 128 ----
    idx_cols = total_padded // 16
    bf16 = _bf16()
    x_bf = np.asarray(x, np.float32).astype(bf16)
    WT_bf = np.ascontiguousarray(np.asarray(W, np.float32).T).astype(bf16)
    iota_bf = np.tile(np.arange(WD, dtype=np.float32), (P, 1)).astype(bf16)
    bvec = np.asarray(b, np.float32).reshape(P, 1)

    in_maps = []
    for k in range(NCORES):
        idx16 = np.ascontiguousarray(ix_pad[k].reshape(idx_cols, 16).T)
        idx128 = np.tile(idx16, (8, 1))
        in_maps.append({
            "x": x_bf,
            "idxs": idx128,
            "consts": np.ascontiguousarray(consts[k]),
            "cbf": iota_bf,
            "wt": WT_bf,
            "bvec": bvec,
        })

    meta = dict(
        T_sc=T_sc, seg_tile0=seg_tile0, T_seg_max=T_seg_max,
        gpad_start=gpad_start, procs=procs, win_nproc=win_nproc,
        n_proc=n_proc, idx_cols=idx_cols,
    )
    return in_maps, meta


def _build_nc(meta, repeat=1, nqueues=4, gbufs=2, ohbufs=8, max_idx=0,
              sp=False):
    from concourse import bacc, mybir
    import concourse.tile as tile

    f32 = mybir.dt.float32
    bf16 = mybir.dt.bfloat16
    i16 = mybir.dt.int16

    T_sc = meta["T_sc"]
    seg_tile0 = meta["seg_tile0"]
    T_seg_max = meta["T_seg_max"]
    gpad_start = meta["gpad_start"]
    procs = meta["procs"]
    win_nproc = meta["win_nproc"]
    n_proc = meta["n_proc"]
    idx_cols = meta["idx_cols"]
    CW = 2 * n_proc

    nc = bacc.Bacc("TRN2", num_swdge_queues=nqueues)
    x_ext = nc.declare_dram_parameter("x", [N_NODES, D], bf16, isOutput=False)
    idx_ext = nc.declare_dram_parameter("idxs", [P, idx_cols], i16, isOutput=False)
    c_ext = nc.declare_dram_parameter("consts", [P, CW], f32, isOutput=False)
    cbf_ext = nc.declare_dram_parameter("cbf", [P, WD], bf16, isOutput=False)
    wt_ext = nc.declare_dram_parameter("wt", [D, D], bf16, isOutput=False)
    b_ext = nc.declare_dram_parameter("bvec", [P, 1], f32, isOutput=False)
    out_ext = nc.declare_dram_parameter("out", [D, NPAD], f32, isOutput=True)

    with tile.TileContext(nc) as tc:
        with ExitStack() as ctx:
            const = ctx.enter_context(tc.tile_pool(name="const", bufs=1))
            gat = ctx.enter_context(tc.tile_pool(name="gat", bufs=gbufs))
            oh_pool = ctx.enter_context(tc.tile_pool(name="oh", bufs=ohbufs))
            agg = ctx.enter_context(tc.tile_pool(name="agg", bufs=3))
            osb = ctx.enter_context(tc.tile_pool(name="osb", bufs=3))
            psum_a = ctx.enter_context(tc.tile_pool(name="psum_a", bufs=2, space="PSUM"))
            psum_o = ctx.enter_context(tc.tile_pool(name="psum_o", bufs=2, space="PSUM"))

            idx_sb = const.tile([P, idx_cols], i16)
            nc.sync.dma_start(idx_sb[:], idx_ext[:])
            c_sb = const.tile([P, CW], f32)
            nc.sync.dma_start(c_sb[:], c_ext[:])
            iota_sb = const.tile([P, WD], bf16)
            nc.sync.dma_start(iota_sb[:], cbf_ext[:])
            wt_sb = const.tile([D, D], bf16)
            nc.sync.dma_start(wt_sb[:], wt_ext[:])
            b_sb = const.tile([P, 1], f32)
            nc.sync.dma_start(b_sb[:], b_ext[:])

            Copy = mybir.ActivationFunctionType.Copy
            Identity = mybir.ActivationFunctionType.Identity

            qi = 0
            for _rep in range(repeat):
                pj = 0  # processing index
                for s in range(NSEG):
                    msgs = gat.tile([P, T_seg_max * D], bf16, tag="msgs")
                    for c in range(NCHUNK):
                        t_cnt = int(T_sc[s, c])
                        if t_cnt == 0:
                            continue
                        icol0 = int(gpad_start[s * NCHUNK + c]) // 16
                        t0_buf = int(seg_tile0[s, c])
                        tstep = t_cnt if max_idx == 0 else max(1, max_idx // P)
                        for t0 in range(0, t_cnt, tstep):
                            tn = min(tstep, t_cnt - t0)
                            nc.gpsimd.dma_gather(
                                out_ap=msgs[:, (t0_buf + t0) * D:(t0_buf + t0 + tn) * D]
                                .rearrange("p (c d) -> p c d", d=D),
                                in_ap=x_ext[c * CHUNK:min((c + 1) * CHUNK, N_NODES), :],
                                idxs_ap=idx_sb[:, icol0 + t0 * (P // 16):
                                               icol0 + (t0 + tn) * (P // 16)],
                                num_idxs=tn * P,
                                num_idxs_reg=tn * P,
                                elem_size=D,
                                single_packet=sp,
                                queue_num=(qi % nqueues),
                            )
                            qi += 1

                    w_lo = s * SEGW
                    w_hi = min((s + 1) * SEGW, NWIN)
                    for w in range(w_lo, w_hi):
                        np_w = int(win_nproc[w])
                        aggT_ps = psum_a.tile([P, WD], f32, space="PSUM")
                        if np_w == 0:
                            # no edges anywhere for this window: out = bias
                            nc.vector.memset(aggT_ps[:], 0.0)
                        for i in range(np_w):
                            (pw, ps_, pc, pt) = procs[pj]
                            assert pw == w and ps_ == s
                            t_buf = int(seg_tile0[s, pc]) + pt
                            oh = oh_pool.tile([P, WD], bf16)
                            nc.vector.tensor_scalar(
                                out=oh[:],
                                in0=iota_sb[:],
                                scalar1=c_sb[:, pj:pj + 1],
                                scalar2=c_sb[:, n_proc + pj:n_proc + pj + 1],
                                op0=mybir.AluOpType.is_equal,
                                op1=mybir.AluOpType.mult,
                            )
                            nc.tensor.matmul(
                                out=aggT_ps[:],
                                lhsT=msgs[:, t_buf * D:(t_buf + 1) * D],
                                rhs=oh[:],
                                start=(i == 0),
                                stop=(i == np_w - 1),
                            )
                            pj += 1
                        aggT_sb = agg.tile([P, WD], bf16, tag="aggT")
                        nc.scalar.activation(aggT_sb[:], aggT_ps[:], Copy)
                        outT_ps = psum_o.tile([P, WD], f32, space="PSUM")
                        nc.tensor.matmul(
                            out=outT_ps[:],
                            lhsT=wt_sb[:],
                            rhs=aggT_sb[:],
                            start=True, stop=True,
                        )
                        outT_sb = osb.tile([P, WD], f32, tag="outT")
                        nc.scalar.activation(outT_sb[:], outT_ps[:], Identity,
                                             bias=b_sb[:, 0:1])
                        nc.sync.dma_start(
                            out_ext[:, w * WD:(w + 1) * WD],
                            outT_sb[:],
                        )
                assert pj == n_proc

    nc.compile()
    return nc


def run(x, edge_index, W, b, trace=False):
    """Build + run on 8 cores. Returns (out, results)."""
    from concourse.bass_utils import run_bass_kernel_spmd

    in_maps, meta = _host_prep(x, edge_index, W, b)
    nc = _build_nc(meta)
    res = run_bass_kernel_spmd(nc, in_maps, list(range(NCORES)), trace=trace)
    parts = [res.results[k]["out"][:, :NLOC].T for k in range(NCORES)]
    out = np.ascontiguousarray(np.concatenate(parts, axis=0), dtype=np.float32)
    return out, res


def kernel(x, edge_index, W, b):
    out, _ = run(x, edge_index, W, b)
    return out


# ---------------------------------------------------------------------------
# benchmarking: time repeat=R vs repeat=1 NEFFs with device-resident inputs;
# the delta cancels transfers/dispatch and yields per-iteration HW time.
# ---------------------------------------------------------------------------

def _make_callable(nc, in_maps):
    import jax
    import numpy as _np
    from jax.sharding import Mesh, PartitionSpec, NamedSharding
    from jax.experimental.shard_map import shard_map
    from concourse import mybir
    from concourse.bass2jax import (
        _bass_exec_p, install_neuronx_cc_hook, partition_id_tensor,
    )

    install_neuronx_cc_hook()
    n_cores = len(in_maps)
    in_names, out_names, out_avals, zero_outs = [], [], [], []
    for alloc in nc.m.functions[0].allocations:
        if not isinstance(alloc, mybir.MemoryLocationSet):
            continue
        name = alloc.memorylocations[0].name
        if alloc.kind == "ExternalInput":
            if nc.partition_id_tensor is None or name != nc.partition_id_tensor.name:
                in_names.append(name)
        elif alloc.kind == "ExternalOutput":
            out_names.append(name)
            shape = tuple(alloc.tensor_shape)
            dtype = mybir.dt.np(alloc.dtype)
            out_avals.append(jax.core.ShapedArray(shape, dtype))
            zero_outs.append(_np.zeros(shape, dtype))
    n_params = len(in_names)
    all_in_names = in_names + out_names
    if nc.partition_id_tensor is not None:
        all_in_names = all_in_names + [nc.partition_id_tensor.name]

    def _body(*args):
        operands = list(args)
        if nc.partition_id_tensor is not None:
            operands.append(partition_id_tensor())
        outs = _bass_exec_p.bind(
            *operands,
            out_avals=tuple(out_avals),
            in_names=tuple(all_in_names),
            out_names=tuple(out_names),
            lowering_input_output_aliases=(),
            sim_require_finite=True,
            sim_require_nnan=True,
            nc=nc,
        )
        return tuple(outs)

    devices = jax.devices()[:n_cores]
    mesh = Mesh(_np.asarray(devices), ("core",))
    spec = PartitionSpec("core")
    in_specs = (spec,) * (n_params + len(out_names))
    out_specs = (spec,) * len(out_names)
    fn = jax.jit(shard_map(_body, mesh=mesh, in_specs=in_specs,
                           out_specs=out_specs, check_rep=False),
                 keep_unused=True)
    sharding = NamedSharding(mesh, spec)
    dev_in = [
        jax.device_put(
            _np.concatenate([_np.asarray(in_maps[c][nm]) for c in range(n_cores)], axis=0),
            sharding)
        for nm in in_names
    ]
    dev_zero = [
        jax.device_put(_np.zeros((n_cores * z.shape[0], *z.shape[1:]), z.dtype), sharding)
        for z in zero_outs
    ]
    return fn, dev_in, dev_zero


def bench(x, edge_index, W, b, big_repeat=5, iters=6, **build_kw):
    import time
    import jax

    in_maps, meta = _host_prep(x, edge_index, W, b)
    fns = {}
    for R in (1, big_repeat):
        nc = _build_nc(meta, repeat=R, **build_kw)
        fn, dev_in, dev_zero = _make_callable(nc, in_maps)
        outs = fn(*dev_in, *dev_zero)  # compile + warm
        jax.block_until_ready(outs)
        fns[R] = (fn, dev_in, dev_zero)
    times = {1: float("inf"), big_repeat: float("inf")}
    for _ in range(iters):
        for R in (1, big_repeat):
            fn, dev_in, dev_zero = fns[R]
            t0 = time.perf_counter()
            outs = fn(*dev_in, *dev_zero)
            jax.block_until_ready(outs)
            times[R] = min(times[R], time.perf_counter() - t0)
    per_iter_ns = (times[big_repeat] - times[1]) / (big_repeat - 1) * 1e9
    return per_iter_ns, times


# revision 12
# speedup vs baseline: 1.1042x; 1.1042x over previous
"""GCN layer kernel for Trainium2, 8 NeuronCores (SPMD).

Math (see reference):
    deg = scatter_add(ones, row); deg = max(deg, 1)
    norm_e = rsqrt(deg[row_e]) * rsqrt(deg[col_e])
    agg[row_e] += x[col_e] * norm_e
    out = agg @ W.T + b

Device strategy (v3):
  - Shard DESTINATION nodes across 8 cores (12500 each) -> no collective.
  - Edges sorted by (core, window, chunk, dest); window = 512 dest nodes
    (NWIN=25). Per (window, chunk) one dma_gather call fetches x[src]
    rows (fp16, 256B rows); groups padded to 128 edges.
  - Scatter-add on the TensorEngine:
        aggT[f, d] += msgs[e, f]^T @ onehot[e, d]
    onehot[e, d] = (d == dloc_e) * norm_e built by one fused DVE
    tensor_scalar (is_equal, mult) per 128-edge tile, fp16 (iota 0..511
    exact in fp16), written into slices of a large slab tile (tile-pool
    allocations cost ~1us each on this HW -> slabs amortize them).
  - PSUM tiles are FULL BANKS ([128, 512] f32): the Tile framework
    guards PSUM by bank, so sub-bank tiles serialize accumulation
    chains (~1us/matmul). Full-bank tiles run at full PE speed.
  - Tiles may span window boundaries: boundary tiles are processed once
    per adjacent window with shifted dloc; foreign edges match no iota
    value and contribute zero.
  - Epilogue per window: aggT PSUM -> SBUF fp16 (scalar engine), then
        outT[o, d] = W[o, :] @ aggT[:, d]
    as one N=512 fp16 matmul (lhsT = W.T), bias b[o] folded via
    scalar.activation(Identity, bias=b). Output is stored feature-major
    [D, NPAD]; the host transposes at the end.

Host-side work is limited to index preprocessing (sort/shard/pad, int16
tables, degree/norm coefficients) and final unpadding/concat/transpose.
"""

import numpy as np
from contextlib import ExitStack

N_NODES = 100000
N_EDGES = 1600000
D = 128
NCORES = 8
NLOC = N_NODES // NCORES          # 12500 real dests per core
WD = 512                          # dest window
NWIN = (NLOC + WD - 1) // WD      # 25 windows (12800 padded dests)
NPAD = NWIN * WD                  # 12800
CHUNK = 32768                     # x chunk rows (int16 index range)
NCHUNK = (N_NODES + CHUNK - 1) // CHUNK  # 4
P = 128
OHK = 16                          # one-hot slab size (tiles per slab)


def _fp16():
    return np.dtype(np.float16)


def _host_prep(x, edge_index, W, b):
    """Sort/shard/pad edges; build per-core device arrays + schedule meta."""
    row = np.asarray(edge_index[0], dtype=np.int64)
    col = np.asarray(edge_index[1], dtype=np.int64)

    deg = np.bincount(row, minlength=N_NODES).astype(np.float32)
    deg = np.maximum(deg, 1.0)
    rs = 1.0 / np.sqrt(deg)
    norm = (rs[row] * rs[col]).astype(np.float32)

    core = row // NLOC
    local = row - core * NLOC
    win = local // WD
    chunk = col >> 15
    # sort by (core, window, chunk, local-dest)
    order = np.lexsort((local, chunk, win, core))
    col_s = col[order]
    local_s = local[order]
    norm_s = norm[order]

    gid = ((core * NWIN + win) * NCHUNK + chunk)[order]
    ngroups = NCORES * NWIN * NCHUNK
    counts = np.bincount(gid, minlength=ngroups).reshape(NCORES, NWIN, NCHUNK)
    starts = np.zeros(ngroups + 1, dtype=np.int64)
    np.cumsum(counts.reshape(-1), out=starts[1:])

    # tiles per (win, chunk): max over cores (shared NEFF), 128-edge tiles
    T_wc = np.ceil(counts.max(axis=0) / P).astype(np.int64)  # [NWIN, NCHUNK]
    total_tiles = int(T_wc.sum())
    total_padded = total_tiles * P
    # padded start of each (win, chunk) group (same for all cores)
    gpad_start = np.zeros(NWIN * NCHUNK + 1, dtype=np.int64)
    np.cumsum((T_wc.reshape(-1) * P), out=gpad_start[1:])
    # tile offset of (win, chunk) within the window's msgs buffer
    win_tile0 = np.zeros((NWIN, NCHUNK), dtype=np.int64)
    for w in range(NWIN):
        off = 0
        for c in range(NCHUNK):
            win_tile0[w, c] = off
            off += int(T_wc[w, c])
    T_win = T_wc.sum(axis=1)                     # [NWIN]
    T_win_max = int(T_win.max())

    # ---- per-core padded flats (dest-local, norm, int16 idx) ----
    loc_pad = np.full((NCORES, total_padded), -1.0, np.float32)
    nrm_pad = np.zeros((NCORES, total_padded), np.float32)
    ix_pad = np.zeros((NCORES, total_padded), np.int16)
    for k in range(NCORES):
        for w in range(NWIN):
            for c in range(NCHUNK):
                g = (k * NWIN + w) * NCHUNK + c
                e0, e1 = starts[g], starts[g + 1]
                n = int(e1 - e0)
                p0 = gpad_start[w * NCHUNK + c]
                loc_pad[k, p0:p0 + n] = local_s[e0:e1].astype(np.float32)
                nrm_pad[k, p0:p0 + n] = norm_s[e0:e1]
                ix_pad[k, p0:p0 + n] = (col_s[e0:e1] - (c << 15)).astype(np.int16)

    # ---- processing schedule (same for all cores) ----
    # For window w, chunk c the edges live in group (w, c) only (sorting is
    # window-major), so tile range is simply [0, T_wc[w, c]). Boundary-tile
    # duplication is unnecessary at window granularity because groups are
    # window-pure. (Padding already rounds each group to 128.)
    procs = []  # (w, c, t_in_group)
    win_nproc = np.zeros(NWIN, np.int64)
    for w in range(NWIN):
        n0 = len(procs)
        for c in range(NCHUNK):
            for t in range(int(T_wc[w, c])):
                procs.append((w, c, t))
        win_nproc[w] = len(procs) - n0
    n_proc = len(procs)

    # ---- per-core consts: dloc | nrm per processing column ----
    consts = np.zeros((NCORES, P, 2 * n_proc), np.float32)
    for j, (w, c, t) in enumerate(procs):
        p0 = gpad_start[w * NCHUNK + c] + t * P
        sl = slice(p0, p0 + P)
        consts[:, :, j] = loc_pad[:, sl] - w * WD
        consts[:, :, n_proc + j] = nrm_pad[:, sl]

    # ---- idx table: wrapped in 16 partitions, replicated to 128 ----
    idx_cols = total_padded // 16
    fp16 = _fp16()
    x_f16 = np.asarray(x, np.float32).astype(fp16)
    WT_f16 = np.ascontiguousarray(np.asarray(W, np.float32).T).astype(fp16)
    iota_f16 = np.tile(np.arange(WD, dtype=np.float32), (P, 1)).astype(fp16)
    bvec = np.asarray(b, np.float32).reshape(P, 1)

    in_maps = []
    for k in range(NCORES):
        idx16 = np.ascontiguousarray(ix_pad[k].reshape(idx_cols, 16).T)
        idx128 = np.tile(idx16, (8, 1))
        in_maps.append({
            "x": x_f16,
            "idxs": idx128,
            "consts": np.ascontiguousarray(consts[k]),
            "cbf": iota_f16,
            "wt": WT_f16,
            "bvec": bvec,
        })

    meta = dict(
        T_wc=T_wc, win_tile0=win_tile0, T_win_max=T_win_max,
        gpad_start=gpad_start, procs=procs, win_nproc=win_nproc,
        n_proc=n_proc, idx_cols=idx_cols,
    )
    return in_maps, meta


def _build_nc(meta, repeat=1, nqueues=4, gbufs=3, ohbufs=2, max_idx=0,
              sp=False, parts=("gather", "compute", "epilogue")):
    from concourse import bacc, mybir
    import concourse.tile as tile

    f32 = mybir.dt.float32
    f16 = mybir.dt.float16
    i16 = mybir.dt.int16

    T_wc = meta["T_wc"]
    win_tile0 = meta["win_tile0"]
    T_win_max = meta["T_win_max"]
    gpad_start = meta["gpad_start"]
    procs = meta["procs"]
    win_nproc = meta["win_nproc"]
    n_proc = meta["n_proc"]
    idx_cols = meta["idx_cols"]
    CW = 2 * n_proc

    nc = bacc.Bacc("TRN2", num_swdge_queues=nqueues)
    x_ext = nc.declare_dram_parameter("x", [N_NODES, D], f16, isOutput=False)
    idx_ext = nc.declare_dram_parameter("idxs", [P, idx_cols], i16, isOutput=False)
    c_ext = nc.declare_dram_parameter("consts", [P, CW], f32, isOutput=False)
    cbf_ext = nc.declare_dram_parameter("cbf", [P, WD], f16, isOutput=False)
    wt_ext = nc.declare_dram_parameter("wt", [D, D], f16, isOutput=False)
    b_ext = nc.declare_dram_parameter("bvec", [P, 1], f32, isOutput=False)
    out_ext = nc.declare_dram_parameter("out", [D, NPAD], f32, isOutput=True)

    with tile.TileContext(nc) as tc:
        with ExitStack() as ctx:
            const = ctx.enter_context(tc.tile_pool(name="const", bufs=1))
            gat = ctx.enter_context(tc.tile_pool(name="gat", bufs=gbufs))
            oh_pool = ctx.enter_context(tc.tile_pool(name="oh", bufs=ohbufs))
            agg = ctx.enter_context(tc.tile_pool(name="agg", bufs=2))
            osb = ctx.enter_context(tc.tile_pool(name="osb", bufs=2))
            psum_a = ctx.enter_context(tc.tile_pool(name="psum_a", bufs=2, space="PSUM"))
            psum_o = ctx.enter_context(tc.tile_pool(name="psum_o", bufs=2, space="PSUM"))

            idx_sb = const.tile([P, idx_cols], i16)
            nc.sync.dma_start(idx_sb[:], idx_ext[:])
            c_sb = const.tile([P, CW], f32)
            nc.sync.dma_start(c_sb[:], c_ext[:])
            iota_sb = const.tile([P, WD], f16)
            nc.sync.dma_start(iota_sb[:], cbf_ext[:])
            wt_sb = const.tile([D, D], f16)
            nc.sync.dma_start(wt_sb[:], wt_ext[:])
            b_sb = const.tile([P, 1], f32)
            nc.sync.dma_start(b_sb[:], b_ext[:])

            Copy = mybir.ActivationFunctionType.Copy
            Identity = mybir.ActivationFunctionType.Identity

            fake_msgs = None
            if "fakegather" in parts:
                fake_msgs = const.tile([P, T_win_max * D], f16)
                nc.sync.dma_start(
                    fake_msgs[:],
                    x_ext[0:P * T_win_max, :].rearrange("(p t) d -> p (t d)", p=P))

            qi = 0
            for _rep in range(repeat):
                pj = 0  # processing index
                oh_slab = None
                oh_used = OHK
                for w in range(NWIN):
                    if fake_msgs is not None:
                        msgs = fake_msgs
                    else:
                        msgs = gat.tile([P, T_win_max * D], f16, tag="msgs")
                    for c in range(NCHUNK):
                        t_cnt = int(T_wc[w, c])
                        if t_cnt == 0 or "gather" not in parts:
                            continue
                        icol0 = int(gpad_start[w * NCHUNK + c]) // 16
                        t0_buf = int(win_tile0[w, c])
                        tstep = t_cnt if max_idx == 0 else max(1, max_idx // P)
                        for t0 in range(0, t_cnt, tstep):
                            tn = min(tstep, t_cnt - t0)
                            nc.gpsimd.dma_gather(
                                out_ap=msgs[:, (t0_buf + t0) * D:(t0_buf + t0 + tn) * D]
                                .rearrange("p (c d) -> p c d", d=D),
                                in_ap=x_ext[c * CHUNK:min((c + 1) * CHUNK, N_NODES), :],
                                idxs_ap=idx_sb[:, icol0 + t0 * (P // 16):
                                               icol0 + (t0 + tn) * (P // 16)],
                                num_idxs=tn * P,
                                num_idxs_reg=tn * P,
                                elem_size=D,
                                single_packet=sp,
                                queue_num=(qi % nqueues),
                            )
                            qi += 1

                    do_oh = "compute" in parts or "onehot" in parts
                    do_mm = "compute" in parts or "matmul" in parts
                    np_w = int(win_nproc[w])
                    if not (do_oh or do_mm or "epilogue" in parts):
                        pj += np_w
                        continue
                    aggT_ps = psum_a.tile([P, WD], f32, space="PSUM")
                    if np_w == 0 or not do_mm:
                        nc.vector.memset(aggT_ps[:], 0.0)
                    for i in range(np_w):
                        (pw, pc, pt) = procs[pj]
                        assert pw == w
                        t_buf = int(win_tile0[w, pc]) + pt
                        if oh_used == OHK:
                            oh_slab = oh_pool.tile([P, OHK * WD], f16, tag="ohs")
                            oh_used = 0
                        oh = oh_slab[:, oh_used * WD:(oh_used + 1) * WD]
                        oh_used += 1
                        if do_oh:
                            nc.vector.tensor_scalar(
                                out=oh,
                                in0=iota_sb[:],
                                scalar1=c_sb[:, pj:pj + 1],
                                scalar2=c_sb[:, n_proc + pj:n_proc + pj + 1],
                                op0=mybir.AluOpType.is_equal,
                                op1=mybir.AluOpType.mult,
                            )
                        if do_mm:
                            nc.tensor.matmul(
                                out=aggT_ps[:],
                                lhsT=msgs[:, t_buf * D:(t_buf + 1) * D],
                                rhs=oh,
                                start=(i == 0),
                                stop=(i == np_w - 1),
                            )
                        pj += 1
                    if "epilogue" not in parts:
                        continue
                    aggT_sb = agg.tile([P, WD], f16, tag="aggT")
                    nc.scalar.activation(aggT_sb[:], aggT_ps[:], Copy)
                    outT_ps = psum_o.tile([P, WD], f32, space="PSUM")
                    nc.tensor.matmul(
                        out=outT_ps[:],
                        lhsT=wt_sb[:],
                        rhs=aggT_sb[:],
                        start=True, stop=True,
                    )
                    outT_sb = osb.tile([P, WD], f32, tag="outT")
                    nc.scalar.activation(outT_sb[:], outT_ps[:], Identity,
                                         bias=b_sb[:, 0:1])
                    nc.sync.dma_start(
                        out_ext[:, w * WD:(w + 1) * WD],
                        outT_sb[:],
                    )
                if "compute" in parts:
                    assert pj == n_proc

    nc.compile()
    return nc


def run(x, edge_index, W, b, trace=False):
    """Build + run on 8 cores. Returns (out, results)."""
    from concourse.bass_utils import run_bass_kernel_spmd

    in_maps, meta = _host_prep(x, edge_index, W, b)
    nc = _build_nc(meta)
    res = run_bass_kernel_spmd(nc, in_maps, list(range(NCORES)), trace=trace)
    parts = [res.results[k]["out"][:, :NLOC].T for k in range(NCORES)]
    out = np.ascontiguousarray(np.concatenate(parts, axis=0), dtype=np.float32)
    return out, res


def kernel(x, edge_index, W, b):
    out, _ = run(x, edge_index, W, b)
    return out


# ---------------------------------------------------------------------------
# benchmarking: time repeat=R vs repeat=1 NEFFs with device-resident inputs;
# the delta cancels transfers/dispatch and yields per-iteration HW time.
# ---------------------------------------------------------------------------

def _make_callable(nc, in_maps):
    import jax
    import numpy as _np
    from jax.sharding import Mesh, PartitionSpec, NamedSharding
    from jax.experimental.shard_map import shard_map
    from concourse import mybir
    from concourse.bass2jax import (
        _bass_exec_p, install_neuronx_cc_hook, partition_id_tensor,
    )

    install_neuronx_cc_hook()
    n_cores = len(in_maps)
    in_names, out_names, out_avals, zero_outs = [], [], [], []
    for alloc in nc.m.functions[0].allocations:
        if not isinstance(alloc, mybir.MemoryLocationSet):
            continue
        name = alloc.memorylocations[0].name
        if alloc.kind == "ExternalInput":
            if nc.partition_id_tensor is None or name != nc.partition_id_tensor.name:
                in_names.append(name)
        elif alloc.kind == "ExternalOutput":
            out_names.append(name)
            shape = tuple(alloc.tensor_shape)
            dtype = mybir.dt.np(alloc.dtype)
            out_avals.append(jax.core.ShapedArray(shape, dtype))
            zero_outs.append(_np.zeros(shape, dtype))
    n_params = len(in_names)
    all_in_names = in_names + out_names
    if nc.partition_id_tensor is not None:
        all_in_names = all_in_names + [nc.partition_id_tensor.name]

    def _body(*args):
        operands = list(args)
        if nc.partition_id_tensor is not None:
            operands.append(partition_id_tensor())
        outs = _bass_exec_p.bind(
            *operands,
            out_avals=tuple(out_avals),
            in_names=tuple(all_in_names),
            out_names=tuple(out_names),
            lowering_input_output_aliases=(),
            sim_require_finite=True,
            sim_require_nnan=True,
            nc=nc,
        )
        return tuple(outs)

    devices = jax.devices()[:n_cores]
    mesh = Mesh(_np.asarray(devices), ("core",))
    spec = PartitionSpec("core")
    in_specs = (spec,) * (n_params + len(out_names))
    out_specs = (spec,) * len(out_names)
    fn = jax.jit(shard_map(_body, mesh=mesh, in_specs=in_specs,
                           out_specs=out_specs, check_rep=False),
                 keep_unused=True)
    sharding = NamedSharding(mesh, spec)
    dev_in = [
        jax.device_put(
            _np.concatenate([_np.asarray(in_maps[c][nm]) for c in range(n_cores)], axis=0),
            sharding)
        for nm in in_names
    ]
    dev_zero = [
        jax.device_put(_np.zeros((n_cores * z.shape[0], *z.shape[1:]), z.dtype), sharding)
        for z in zero_outs
    ]
    return fn, dev_in, dev_zero


def bench(x, edge_index, W, b, big_repeat=21, iters=8, **build_kw):
    import time
    import jax

    in_maps, meta = _host_prep(x, edge_index, W, b)
    fns = {}
    for R in (1, big_repeat):
        nc = _build_nc(meta, repeat=R, **build_kw)
        fn, dev_in, dev_zero = _make_callable(nc, in_maps)
        outs = fn(*dev_in, *dev_zero)  # compile + warm
        jax.block_until_ready(outs)
        fns[R] = (fn, dev_in, dev_zero)
    times = {1: float("inf"), big_repeat: float("inf")}
    for _ in range(iters):
        for R in (1, big_repeat):
            fn, dev_in, dev_zero = fns[R]
            t0 = time.perf_counter()
            outs = fn(*dev_in, *dev_zero)
            jax.block_until_ready(outs)
            times[R] = min(times[R], time.perf_counter() - t0)
    per_iter_ns = (times[big_repeat] - times[1]) / (big_repeat - 1) * 1e9
    return per_iter_ns, times
